# revision 5
# baseline (speedup 1.0000x reference)
"""Trainium2 Bass kernel for nn_NodeEncoding_72816875537095.

Reference computation:
    scores = x @ W[0] + b[0]                          # [total]
    sp     = scatter(scores, pad_idx) -> [B, 96]      # padded per-graph scores
    num    = einsum('bijk,bk->bij', paths, sp)
    den    = paths.sum(-1) + 1e-8
    out    = num / den                                # [64, 96, 96]

Key structural fact: paths[b] is zero outside the graph's valid
L_b x L_b x L_b block (L_b in [48, 90]), so only ~41% of the tensor
carries data.  This kernel crops to the valid blocks, cutting HBM
traffic per core from 9.4 MB (padded k-major fp8) to ~3.0 MB.

SPMD layout (one program, 8 cores, per-core data differs):
  - Graphs grouped by length L.  Each core gets one "own" graph per
    length (full block) plus a fixed 1/8 chunk-slice of each leftover
    "shared" graph, zero-padded to a fixed per-slot shape, so every
    core runs an identical instruction stream.
  - Per slot: paths cropped block, k-major [L, nch*128] fp8 (0/1 exact).
  - Scores are computed on the tensor engine: per slot, two k=128
    matmuls xT(bf16) @ Wcol(bf16) accumulate into PSUM [96, NSLOT],
    plus one k=1 ones @ b matmul for the bias.  The small xT DMA is
    issued before the big paths DMAs so scores are ready early.
  - Per 128-column chunk of a slot: one matmul with the paths chunk as
    the fp8 stationary operand [L, 128] and moving operand
    w_all[0:L, 4s:4s+4] = [sp_hi, sp_lo, ones, 0] (fp8, 2-term hi/lo
    split of the scores -> ~4e-4 rel err) -> PSUM [128, 4] =
    (num_hi, num_lo, den, -).
  - 128 chunks per PSUM bank; epilogue per bank: hi copy (scalar),
    num = hi + lo/16 (DVE), den + eps (scalar), reciprocal (DVE),
    multiply (DVE).  Output stored partition-major [128, NCH]; host
    scatters chunks back into [64, 96, 96].
"""

import sys

if "/opt/trn_rl_repo" not in sys.path:
    sys.path.insert(0, "/opt/trn_rl_repo")

import math

import ml_dtypes
import numpy as np

import concourse.bass as bass  # noqa: F401
import concourse.mybir as mybir
from concourse import bacc, bass_utils
from concourse.tile import TileContext

F32 = mybir.dt.float32
BF16 = mybir.dt.bfloat16
FP8 = mybir.dt.float8e4
AF = mybir.ActivationFunctionType

B = 64
MAX_A = 96
D = 256
N_CORES = 8
CHUNK = 128                 # stationary columns per matmul
CPT = 128                   # chunks per PSUM tile (128*4 = 512 cols = 1 bank)
EPS = 1e-8

NP_FP8 = ml_dtypes.float8_e4m3
NP_BF16 = ml_dtypes.bfloat16

_CACHE = {}


def _make_template(lengths):
    """Build the per-core slot template from the 64 graph lengths.

    Returns (slots, groups) where every core sees identical structure;
    per-core variation lives in which graph / chunk-range fills a slot.
    """
    by_len = {}
    for g, L in enumerate(lengths):
        by_len.setdefault(int(L), []).append(g)

    slots = []  # dicts: l, nch, graphs[8], chunk0[8]
    for L in sorted(by_len, reverse=True):
        gs = by_len[L]
        n_full = math.ceil(L * L / CHUNK)
        n_own = len(gs) // N_CORES
        for j in range(n_own):
            slots.append(dict(
                l=L, nch=n_full, kind="own",
                graphs=[gs[j * N_CORES + c] for c in range(N_CORES)],
                chunk0=[0] * N_CORES))
        for gsh in gs[n_own * N_CORES:]:
            m = math.ceil(n_full / N_CORES)
            slots.append(dict(
                l=L, nch=m, kind="shared",
                graphs=[gsh] * N_CORES,
                chunk0=[c * m for c in range(N_CORES)]))

    # Merge slots of equal L into one DMA group; groups ordered by
    # descending byte size so the big transfers stream first.
    groups = []  # dicts: l, slot_ids, cols
    for L in sorted(by_len, reverse=True):
        sids = [i for i, s in enumerate(slots) if s["l"] == L]
        cols = sum(slots[i]["nch"] * CHUNK for i in sids)
        groups.append(dict(l=L, slot_ids=sids, cols=cols))
    groups.sort(key=lambda gr: -gr["l"] * gr["cols"])

    # Re-order slots into group order so matmul emission order (= PSUM
    # chunk order) matches DMA arrival order.
    order = [i for gr in groups for i in gr["slot_ids"]]
    slots = [slots[i] for i in order]
    for gr in groups:
        gr["slot_ids"] = None  # re-derive below
    pos = 0
    for gr in groups:
        n = sum(1 for s in slots if s["l"] == gr["l"])
        gr["slot_ids"] = list(range(pos, pos + n))
        pos += n

    # Column base of each slot inside its group tile, chunk base in the
    # global PSUM/output ordering, and xT column offsets.
    chunk_base = []
    acc = 0
    for s in slots:
        chunk_base.append(acc)
        acc += s["nch"]
    nch_tot = acc

    colbase = {}
    for gr in groups:
        cacc = 0
        for i in gr["slot_ids"]:
            colbase[i] = cacc
            cacc += slots[i]["nch"] * CHUNK

    xt_off = []
    xacc = 0
    for s in slots:
        xt_off.append(xacc)
        xacc += 2 * s["l"]
    xt_cols = xacc

    return dict(slots=slots, groups=groups, chunk_base=chunk_base,
                nch_tot=nch_tot, colbase=colbase, xt_off=xt_off,
                xt_cols=xt_cols)


def _build(lengths):
    key = tuple(int(v) for v in lengths)
    if key in _CACHE:
        return _CACHE[key]

    tmpl = _make_template(lengths)
    slots = tmpl["slots"]
    groups = tmpl["groups"]
    nslot = len(slots)
    nch_tot = tmpl["nch_tot"]

    nc = bacc.Bacc("TRN2", target_bir_lowering=False, debug=False,
                   num_devices=N_CORES)

    pg_d = [nc.dram_tensor(f"pg{i}", [gr["l"], gr["cols"]], FP8,
                           kind="ExternalInput")
            for i, gr in enumerate(groups)]
    xt_d = nc.dram_tensor("xT", [D // 2, tmpl["xt_cols"]], BF16,
                          kind="ExternalInput")
    w2_d = nc.dram_tensor("W2", [D // 2, 2], BF16, kind="ExternalInput")
    bv_d = nc.dram_tensor("bvec", [1, nslot], BF16, kind="ExternalInput")
    out_d = nc.dram_tensor("out", [CHUNK, nch_tot], F32,
                           kind="ExternalOutput")

    with TileContext(nc) as tc:
        with (
            tc.tile_pool(name="data", bufs=1) as dpool,
            tc.tile_pool(name="psum", bufs=2, space="PSUM") as pspool,
            tc.tile_pool(name="psc", bufs=1, space="PSUM") as pscp,
            tc.tile_pool(name="epi", bufs=3) as epool,
        ):
            # ---- issue all DMAs up front; xT + tiny score inputs first
            xt = dpool.tile([D // 2, tmpl["xt_cols"]], BF16, name="xt")
            nc.sync.dma_start(out=xt[:], in_=xt_d[:])
            w2 = dpool.tile([D // 2, 2], BF16, name="w2")
            nc.scalar.dma_start(out=w2[:], in_=w2_d[:])
            bv = dpool.tile([1, nslot], BF16, name="bv")
            nc.scalar.dma_start(out=bv[:], in_=bv_d[:])

            gtiles = []
            for i, gr in enumerate(groups):
                gt = dpool.tile([gr["l"], gr["cols"]], FP8, name=f"pg{i}")
                eng = nc.sync if i % 2 == 0 else nc.scalar
                eng.dma_start(out=gt[:], in_=pg_d[i][:])
                gtiles.append(gt)

            # ---- scores on the tensor engine -> PSUM [96, nslot]
            ones = dpool.tile([1, MAX_A], BF16, name="ones")
            nc.vector.memset(ones[:], 1.0)
            ps_sc = pscp.tile([MAX_A, nslot], F32, name="ps_sc")
            nc.tensor.matmul(ps_sc[:], lhsT=ones[:], rhs=bv[:],
                             start=True, stop=False)
            for s, sl in enumerate(slots):
                q, l = tmpl["xt_off"][s], sl["l"]
                nc.tensor.matmul(ps_sc[0:l, s:s + 1],
                                 lhsT=xt[:, q:q + l], rhs=w2[:, 0:1],
                                 start=False, stop=False)
                nc.tensor.matmul(ps_sc[0:l, s:s + 1],
                                 lhsT=xt[:, q + l:q + 2 * l],
                                 rhs=w2[:, 1:2],
                                 start=False, stop=(s == nslot - 1))

            # ---- w_all [96, 4*nslot] fp8: [sp_hi, sp_lo*16, ones, 0]
            w_hi = dpool.tile([MAX_A, nslot], FP8, name="w_hi")
            nc.vector.tensor_copy(w_hi[:], ps_sc[:])
            r1 = dpool.tile([MAX_A, nslot], F32, name="r1")
            nc.vector.tensor_tensor(out=r1[:], in0=ps_sc[:], in1=w_hi[:],
                                    op=mybir.AluOpType.subtract)
            w_lo = dpool.tile([MAX_A, nslot], FP8, name="w_lo")
            nc.vector.tensor_scalar_mul(out=w_lo[:], in0=r1[:],
                                        scalar1=16.0)
            w_all = dpool.tile([MAX_A, 4 * nslot], FP8, name="w_all")
            nc.vector.memset(w_all[:], 0.0)
            nc.vector.memset(w_all[:, 2:4 * nslot:4], 1.0)
            nc.vector.tensor_copy(w_all[:, 0:4 * nslot:4], w_hi[:])
            nc.vector.tensor_copy(w_all[:, 1:4 * nslot:4], w_lo[:])

            out_sb = dpool.tile([CHUNK, nch_tot], F32, name="out_sb")

            # ---- main loop: one matmul per 128-column chunk
            ps = None
            n_in_tile = 0
            gidx = 0
            for gi, gr in enumerate(groups):
                for s in gr["slot_ids"]:
                    sl = slots[s]
                    gt = gtiles[gi]
                    cb = tmpl["colbase"][s]
                    l = sl["l"]
                    for c in range(sl["nch"]):
                        r = gidx % CPT
                        if r == 0:
                            n_in_tile = min(CPT, nch_tot - gidx)
                            ps = pspool.tile([CHUNK, 4 * n_in_tile], F32,
                                             tag="ps")
                        col = cb + CHUNK * c
                        nc.tensor.matmul(
                            ps[:, 4 * r:4 * r + 4],
                            lhsT=gt[:, col:col + CHUNK],
                            rhs=w_all[0:l, 4 * s:4 * s + 4],
                            start=True, stop=True)
                        if r == n_in_tile - 1:
                            t0 = gidx // CPT
                            w = n_in_tile
                            hi_sb = epool.tile([CHUNK, CPT], F32, tag="hi")
                            nc.scalar.activation(
                                out=hi_sb[:, :w], in_=ps[:, 0:4 * w:4],
                                func=AF.Copy)
                            numt = epool.tile([CHUNK, CPT], F32,
                                              tag="numt")
                            nc.vector.scalar_tensor_tensor(
                                out=numt[:, :w], in0=ps[:, 1:4 * w:4],
                                scalar=0.0625, in1=hi_sb[:, :w],
                                op0=mybir.AluOpType.mult,
                                op1=mybir.AluOpType.add)
                            den_sb = epool.tile([CHUNK, CPT], F32,
                                                tag="den")
                            nc.scalar.activation(
                                out=den_sb[:, :w], in_=ps[:, 2:4 * w:4],
                                func=AF.Copy, bias=EPS)
                            rec = epool.tile([CHUNK, CPT], F32, tag="rec")
                            nc.vector.reciprocal(out=rec[:, :w],
                                                 in_=den_sb[:, :w])
                            nc.vector.tensor_tensor(
                                out=out_sb[:, CPT * t0:CPT * t0 + w],
                                in0=numt[:, :w], in1=rec[:, :w],
                                op=mybir.AluOpType.mult)
                        gidx += 1

            nc.sync.dma_start(out=out_d[:], in_=out_sb[:])

    nc.compile()
    _CACHE[key] = (nc, tmpl)
    return nc, tmpl


def _host_prep(x, W, b, paths, lengths, offsets, tmpl):
    slots = tmpl["slots"]
    groups = tmpl["groups"]
    nslot = len(slots)

    # k-major cropped fp8 block per graph, computed once
    kmajor = {}
    for g, L in enumerate(lengths):
        if any(g in s["graphs"] for s in slots):
            blk = np.asarray(paths[g, :L, :L, :L], dtype=np.float32)
            kmajor[g] = np.ascontiguousarray(
                blk.transpose(2, 0, 1).reshape(L, L * L)).astype(NP_FP8)

    xb = np.asarray(x, dtype=np.float32).astype(NP_BF16)
    # W2[:, 0] = W[0:128], W2[:, 1] = W[128:256]
    w2 = np.ascontiguousarray(
        np.stack([np.asarray(W, np.float32)[0, :D // 2],
                  np.asarray(W, np.float32)[0, D // 2:]], axis=1)
        .astype(NP_BF16))
    bvec = np.full((1, nslot), float(np.asarray(b)[0]), dtype=NP_BF16)

    in_maps = []
    for core in range(N_CORES):
        im = {"W2": w2, "bvec": bvec}
        # xT: per slot [128, 2L] = [x[:,0:128].T | x[:,128:256].T]
        xt = np.zeros((D // 2, tmpl["xt_cols"]), dtype=NP_BF16)
        for s, sl in enumerate(slots):
            g, L, q = sl["graphs"][core], sl["l"], tmpl["xt_off"][s]
            xg = xb[offsets[g]:offsets[g] + L]  # [L, 256]
            xt[:, q:q + L] = xg[:, :D // 2].T
            xt[:, q + L:q + 2 * L] = xg[:, D // 2:].T
        im["xT"] = xt

        for i, gr in enumerate(groups):
            arr = np.zeros((gr["l"], gr["cols"]), dtype=NP_FP8)
            for s in gr["slot_ids"]:
                sl = slots[s]
                g, L = sl["graphs"][core], sl["l"]
                cb = tmpl["colbase"][s]
                c0 = sl["chunk0"][core] * CHUNK
                c1 = min(L * L, c0 + sl["nch"] * CHUNK)
                if c1 > c0:
                    arr[:, cb:cb + (c1 - c0)] = kmajor[g][:, c0:c1]
            im[f"pg{i}"] = arr
        in_maps.append(im)
    return in_maps


LAST_RESULTS = None


def kernel(x, W, b, paths, pad_idx, _trace=False):
    global LAST_RESULTS
    pad_idx = np.asarray(pad_idx)
    lengths = np.bincount(pad_idx // MAX_A, minlength=B).astype(np.int64)
    offsets = np.zeros(B + 1, dtype=np.int64)
    np.cumsum(lengths, out=offsets[1:])

    nc, tmpl = _build(lengths)
    in_maps = _host_prep(x, W, b, paths, lengths, offsets, tmpl)
    res = bass_utils.run_bass_kernel_spmd(
        nc, in_maps, core_ids=list(range(N_CORES)), trace=_trace)
    LAST_RESULTS = res

    slots = tmpl["slots"]
    out = np.zeros((B, MAX_A, MAX_A), dtype=np.float32)
    flat = {g: np.zeros(int(L) * int(L), dtype=np.float32)
            for g, L in enumerate(lengths)}
    for core in range(N_CORES):
        oc = res.results[core]["out"]  # [128, nch_tot] partition-major
        for s, sl in enumerate(slots):
            g, L = sl["graphs"][core], sl["l"]
            cb = tmpl["chunk_base"][s]
            c0 = sl["chunk0"][core] * CHUNK
            c1 = min(L * L, c0 + sl["nch"] * CHUNK)
            if c1 > c0:
                vals = oc[:, cb:cb + sl["nch"]].T.reshape(-1)[:c1 - c0]
                flat[g][c0:c1] = vals
    for g, L in enumerate(lengths):
        L = int(L)
        out[g, :L, :L] = flat[g].reshape(L, L)
    return out


# revision 12
# speedup vs baseline: 1.6950x; 1.6950x over previous
"""Trainium2 Bass kernel for nn_NodeEncoding_72816875537095.

Reference computation:
    scores = x @ W[0] + b[0]                          # [total]
    sp     = scatter(scores, pad_idx) -> [B, 96]      # padded per-graph scores
    num    = einsum('bijk,bk->bij', paths, sp)
    den    = paths.sum(-1) + 1e-8
    out    = num / den                                # [64, 96, 96]

Key structural fact: paths[b] is zero outside the graph's valid
L_b x L_b x L_b block (L_b in [48, 90]), so only ~41% of the tensor
carries data.  This kernel crops to the valid blocks, cutting HBM
traffic per core from 9.4 MB (padded k-major fp8) to ~3.0 MB.

SPMD layout (one program, 8 cores, per-core data differs):
  - Graphs grouped by length L.  Each core gets one "own" graph per
    length (full block) plus a fixed 1/8 chunk-slice of each leftover
    "shared" graph, zero-padded to a fixed per-slot shape, so every
    core runs an identical instruction stream.
  - Per slot: paths cropped block, k-major [L, nch*128] fp8 (0/1 exact).
  - Scores are computed on the tensor engine: per slot, two k=128
    matmuls xT(bf16) @ Wcol(bf16) accumulate into PSUM [96, NSLOT],
    plus one k=1 ones @ b matmul for the bias.  The small xT DMA is
    issued before the big paths DMAs so scores are ready early.
  - Per 128-column chunk of a slot: one matmul with the paths chunk as
    the fp8 stationary operand [L, 128] and moving operand
    w_all[0:L, 4s:4s+4] = [sp_hi, sp_lo, ones, 0] (fp8, 2-term hi/lo
    split of the scores -> ~4e-4 rel err) -> PSUM [128, 4] =
    (num_hi, num_lo, den, -).
  - 128 chunks per PSUM bank; epilogue per bank: hi copy (scalar),
    num = hi + lo/16 (DVE), den + eps (scalar), reciprocal (DVE),
    multiply (DVE).  Output stored partition-major [128, NCH]; host
    scatters chunks back into [64, 96, 96].
"""

import sys

if "/opt/trn_rl_repo" not in sys.path:
    sys.path.insert(0, "/opt/trn_rl_repo")

import math

import ml_dtypes
import numpy as np

import concourse.bass as bass  # noqa: F401
import concourse.mybir as mybir
from concourse import bacc, bass_utils
from concourse.tile import TileContext

F32 = mybir.dt.float32
BF16 = mybir.dt.bfloat16
FP8 = mybir.dt.float8e4
AF = mybir.ActivationFunctionType

B = 64
MAX_A = 96
D = 256
N_CORES = 8
CHUNK = 128                 # stationary columns per matmul
CPT = 128                   # chunks per PSUM tile (128*4 = 512 cols = 1 bank)
EPS = 1e-8

NP_FP8 = ml_dtypes.float8_e4m3
NP_BF16 = ml_dtypes.bfloat16

_CACHE = {}


def _make_template(lengths):
    """Build the per-core slot template from the 64 graph lengths.

    Returns (slots, groups) where every core sees identical structure;
    per-core variation lives in which graph / chunk-range fills a slot.
    """
    by_len = {}
    for g, L in enumerate(lengths):
        by_len.setdefault(int(L), []).append(g)

    slots = []  # dicts: l, nch, graphs[8], chunk0[8]
    for L in sorted(by_len, reverse=True):
        gs = by_len[L]
        n_full = math.ceil(L * L / CHUNK)
        n_own = len(gs) // N_CORES
        for j in range(n_own):
            slots.append(dict(
                l=L, nch=n_full, kind="own",
                graphs=[gs[j * N_CORES + c] for c in range(N_CORES)],
                chunk0=[0] * N_CORES))
        for gsh in gs[n_own * N_CORES:]:
            m = math.ceil(n_full / N_CORES)
            slots.append(dict(
                l=L, nch=m, kind="shared",
                graphs=[gsh] * N_CORES,
                chunk0=[c * m for c in range(N_CORES)]))

    # Merge slots of equal L into one DMA group; groups ordered by
    # descending byte size so the big transfers stream first.  The DMA
    # partition count is padded to a multiple of 16: the HW DGE splits a
    # 2D transfer across (largest divisor of row count <= 16) queues, so
    # e.g. an 83-row DMA lands on a single queue at ~19 GB/s.
    groups = []  # dicts: l, lpad, slot_ids, cols
    for L in sorted(by_len, reverse=True):
        sids = [i for i, s in enumerate(slots) if s["l"] == L]
        cols = sum(slots[i]["nch"] * CHUNK for i in sids)
        groups.append(dict(l=L, lpad=16 * math.ceil(L / 16),
                           slot_ids=sids, cols=cols))
    groups.sort(key=lambda gr: -gr["l"] * gr["cols"])

    # Re-order slots into group order so matmul emission order (= PSUM
    # chunk order) matches DMA arrival order.
    order = [i for gr in groups for i in gr["slot_ids"]]
    slots = [slots[i] for i in order]
    for gr in groups:
        gr["slot_ids"] = None  # re-derive below
    pos = 0
    for gr in groups:
        n = sum(1 for s in slots if s["l"] == gr["l"])
        gr["slot_ids"] = list(range(pos, pos + n))
        pos += n

    # Column base of each slot inside its group tile, chunk base in the
    # global PSUM/output ordering, and xT column offsets.
    chunk_base = []
    acc = 0
    for s in slots:
        chunk_base.append(acc)
        acc += s["nch"]
    nch_tot = acc

    colbase = {}
    for gr in groups:
        cacc = 0
        for i in gr["slot_ids"]:
            colbase[i] = cacc
            cacc += slots[i]["nch"] * CHUNK

    xt_off = []
    xacc = 0
    for s in slots:
        xt_off.append(xacc)
        xacc += 2 * s["l"]
    # W halves and the bias row-vector ride in the same tensor (their
    # own DMAs would be 4-byte-per-row transfers).
    w2_off = xacc
    bv_off = xacc + 2
    xt_cols = 16 * math.ceil((bv_off + len(slots)) / 16)

    return dict(slots=slots, groups=groups, chunk_base=chunk_base,
                nch_tot=nch_tot, colbase=colbase, xt_off=xt_off,
                xt_cols=xt_cols, w2_off=w2_off, bv_off=bv_off)


def _build(lengths):
    key = tuple(int(v) for v in lengths)
    if key in _CACHE:
        return _CACHE[key]

    tmpl = _make_template(lengths)
    slots = tmpl["slots"]
    groups = tmpl["groups"]
    nslot = len(slots)
    nch_tot = tmpl["nch_tot"]

    nc = bacc.Bacc("TRN2", target_bir_lowering=False, debug=False,
                   num_devices=N_CORES)
    # The SWDGE ring (gpsimd DMAs) is unused: drop its declaration so the
    # runtime's per-queue setup/teardown sync ladder is 16 queues shorter.
    nc.m.queues = [q for q in nc.m.queues if q.name != "qPoolDynamic"]

    pg_d = [nc.dram_tensor(f"pg{i}", [gr["lpad"], gr["cols"]], FP8,
                           kind="ExternalInput")
            for i, gr in enumerate(groups)]
    xt_d = nc.dram_tensor("xT", [D // 2, tmpl["xt_cols"]], BF16,
                          kind="ExternalInput")
    out_d = nc.dram_tensor("out", [CHUNK, nch_tot], F32,
                           kind="ExternalOutput")

    with TileContext(nc) as tc:
        with (
            tc.tile_pool(name="data", bufs=1) as dpool,
            tc.tile_pool(name="psum", bufs=2, space="PSUM") as pspool,
            tc.tile_pool(name="psc", bufs=1, space="PSUM") as pscp,
            tc.tile_pool(name="epi", bufs=3) as epool,
        ):
            # ---- issue all DMAs up front; xT (scores inputs) first
            xt = dpool.tile([D // 2, tmpl["xt_cols"]], BF16, name="xt")
            nc.sync.dma_start(out=xt[:], in_=xt_d[:])

            gtiles = []
            for i, gr in enumerate(groups):
                gt = dpool.tile([gr["lpad"], gr["cols"]], FP8,
                                name=f"pg{i}")
                eng = nc.sync if i % 2 == 0 else nc.scalar
                eng.dma_start(out=gt[:], in_=pg_d[i][:])
                gtiles.append(gt)

            # ---- scores on the tensor engine -> PSUM [96, nslot]
            w2o, bvo = tmpl["w2_off"], tmpl["bv_off"]
            ones = dpool.tile([1, MAX_A], BF16, name="ones")
            nc.vector.memset(ones[:], 1.0)
            ps_sc = pscp.tile([MAX_A, nslot], F32, name="ps_sc")
            nc.tensor.matmul(ps_sc[:], lhsT=ones[:],
                             rhs=xt[0:1, bvo:bvo + nslot],
                             start=True, stop=False)
            for s, sl in enumerate(slots):
                q, l = tmpl["xt_off"][s], sl["l"]
                nc.tensor.matmul(ps_sc[0:l, s:s + 1],
                                 lhsT=xt[:, q:q + l],
                                 rhs=xt[:, w2o:w2o + 1],
                                 start=False, stop=False)
                nc.tensor.matmul(ps_sc[0:l, s:s + 1],
                                 lhsT=xt[:, q + l:q + 2 * l],
                                 rhs=xt[:, w2o + 1:w2o + 2],
                                 start=False, stop=(s == nslot - 1))

            # ---- w_all [96, 4*nslot] fp8: [sp_hi, sp_lo*16, ones, 0]
            w_hi = dpool.tile([MAX_A, nslot], FP8, name="w_hi")
            nc.vector.tensor_copy(w_hi[:], ps_sc[:])
            r1 = dpool.tile([MAX_A, nslot], F32, name="r1")
            nc.vector.tensor_tensor(out=r1[:], in0=ps_sc[:], in1=w_hi[:],
                                    op=mybir.AluOpType.subtract)
            w_lo = dpool.tile([MAX_A, nslot], FP8, name="w_lo")
            nc.vector.tensor_scalar_mul(out=w_lo[:], in0=r1[:],
                                        scalar1=16.0)
            w_all = dpool.tile([MAX_A, 4 * nslot], FP8, name="w_all")
            nc.vector.memset(w_all[:], 0.0)
            nc.vector.memset(w_all[:, 2:4 * nslot:4], 1.0)
            nc.vector.tensor_copy(w_all[:, 0:4 * nslot:4], w_hi[:])
            nc.vector.tensor_copy(w_all[:, 1:4 * nslot:4], w_lo[:])

            out_sb = dpool.tile([CHUNK, nch_tot], F32, name="out_sb")

            # ---- main loop: one matmul per 128-column chunk
            ps = None
            n_in_tile = 0
            gidx = 0
            for gi, gr in enumerate(groups):
                for s in gr["slot_ids"]:
                    sl = slots[s]
                    gt = gtiles[gi]
                    cb = tmpl["colbase"][s]
                    l = sl["l"]
                    for c in range(sl["nch"]):
                        r = gidx % CPT
                        if r == 0:
                            n_in_tile = min(CPT, nch_tot - gidx)
                            ps = pspool.tile([CHUNK, 4 * n_in_tile], F32,
                                             tag="ps")
                        col = cb + CHUNK * c
                        nc.tensor.matmul(
                            ps[:, 4 * r:4 * r + 4],
                            lhsT=gt[0:l, col:col + CHUNK],
                            rhs=w_all[0:l, 4 * s:4 * s + 4],
                            start=True, stop=True)
                        if r == n_in_tile - 1:
                            t0 = gidx // CPT
                            w = n_in_tile
                            hi_sb = epool.tile([CHUNK, CPT], F32, tag="hi")
                            nc.scalar.activation(
                                out=hi_sb[:, :w], in_=ps[:, 0:4 * w:4],
                                func=AF.Copy)
                            numt = epool.tile([CHUNK, CPT], F32,
                                              tag="numt")
                            nc.vector.scalar_tensor_tensor(
                                out=numt[:, :w], in0=ps[:, 1:4 * w:4],
                                scalar=0.0625, in1=hi_sb[:, :w],
                                op0=mybir.AluOpType.mult,
                                op1=mybir.AluOpType.add)
                            den_sb = epool.tile([CHUNK, CPT], F32,
                                                tag="den")
                            nc.scalar.activation(
                                out=den_sb[:, :w], in_=ps[:, 2:4 * w:4],
                                func=AF.Copy, bias=EPS)
                            rec = epool.tile([CHUNK, CPT], F32, tag="rec")
                            nc.vector.reciprocal(out=rec[:, :w],
                                                 in_=den_sb[:, :w])
                            nc.vector.tensor_tensor(
                                out=out_sb[:, CPT * t0:CPT * t0 + w],
                                in0=numt[:, :w], in1=rec[:, :w],
                                op=mybir.AluOpType.mult)
                        gidx += 1

            nc.sync.dma_start(out=out_d[:], in_=out_sb[:])

    nc.compile()
    _CACHE[key] = (nc, tmpl)
    return nc, tmpl


def _host_prep(x, W, b, paths, lengths, offsets, tmpl):
    slots = tmpl["slots"]
    groups = tmpl["groups"]
    nslot = len(slots)

    # k-major cropped fp8 block per graph, computed once
    kmajor = {}
    for g, L in enumerate(lengths):
        if any(g in s["graphs"] for s in slots):
            blk = np.asarray(paths[g, :L, :L, :L], dtype=np.float32)
            kmajor[g] = np.ascontiguousarray(
                blk.transpose(2, 0, 1).reshape(L, L * L)).astype(NP_FP8)

    xb = np.asarray(x, dtype=np.float32).astype(NP_BF16)
    wf = np.asarray(W, np.float32)

    in_maps = []
    for core in range(N_CORES):
        im = {}
        # xT: per slot [128, 2L] = [x[:,0:128].T | x[:,128:256].T],
        # then the two W halves as columns and the bias as a row vector.
        xt = np.zeros((D // 2, tmpl["xt_cols"]), dtype=NP_BF16)
        for s, sl in enumerate(slots):
            g, L, q = sl["graphs"][core], sl["l"], tmpl["xt_off"][s]
            xg = xb[offsets[g]:offsets[g] + L]  # [L, 256]
            xt[:, q:q + L] = xg[:, :D // 2].T
            xt[:, q + L:q + 2 * L] = xg[:, D // 2:].T
        xt[:, tmpl["w2_off"]] = wf[0, :D // 2]
        xt[:, tmpl["w2_off"] + 1] = wf[0, D // 2:]
        xt[0, tmpl["bv_off"]:tmpl["bv_off"] + nslot] = float(
            np.asarray(b)[0])
        im["xT"] = xt

        for i, gr in enumerate(groups):
            arr = np.zeros((gr["lpad"], gr["cols"]), dtype=NP_FP8)
            for s in gr["slot_ids"]:
                sl = slots[s]
                g, L = sl["graphs"][core], sl["l"]
                cb = tmpl["colbase"][s]
                c0 = sl["chunk0"][core] * CHUNK
                c1 = min(L * L, c0 + sl["nch"] * CHUNK)
                if c1 > c0:
                    arr[:L, cb:cb + (c1 - c0)] = kmajor[g][:, c0:c1]
            im[f"pg{i}"] = arr
        in_maps.append(im)
    return in_maps


LAST_RESULTS = None


def kernel(x, W, b, paths, pad_idx, _trace=False):
    global LAST_RESULTS
    pad_idx = np.asarray(pad_idx)
    lengths = np.bincount(pad_idx // MAX_A, minlength=B).astype(np.int64)
    offsets = np.zeros(B + 1, dtype=np.int64)
    np.cumsum(lengths, out=offsets[1:])

    nc, tmpl = _build(lengths)
    in_maps = _host_prep(x, W, b, paths, lengths, offsets, tmpl)
    res = bass_utils.run_bass_kernel_spmd(
        nc, in_maps, core_ids=list(range(N_CORES)), trace=_trace)
    LAST_RESULTS = res

    slots = tmpl["slots"]
    out = np.zeros((B, MAX_A, MAX_A), dtype=np.float32)
    flat = {g: np.zeros(int(L) * int(L), dtype=np.float32)
            for g, L in enumerate(lengths)}
    for core in range(N_CORES):
        oc = res.results[core]["out"]  # [128, nch_tot] partition-major
        for s, sl in enumerate(slots):
            g, L = sl["graphs"][core], sl["l"]
            cb = tmpl["chunk_base"][s]
            c0 = sl["chunk0"][core] * CHUNK
            c1 = min(L * L, c0 + sl["nch"] * CHUNK)
            if c1 > c0:
                vals = oc[:, cb:cb + sl["nch"]].T.reshape(-1)[:c1 - c0]
                flat[g][c0:c1] = vals
    for g, L in enumerate(lengths):
        L = int(L)
        out[g, :L, :L] = flat[g].reshape(L, L)
    return out


# revision 15
# speedup vs baseline: 1.7115x; 1.0097x over previous
"""Trainium2 Bass kernel for nn_NodeEncoding_72816875537095.

Reference computation:
    scores = x @ W[0] + b[0]                          # [total]
    sp     = scatter(scores, pad_idx) -> [B, 96]      # padded per-graph scores
    num    = einsum('bijk,bk->bij', paths, sp)
    den    = paths.sum(-1) + 1e-8
    out    = num / den                                # [64, 96, 96]

Key structural fact: paths[b] is zero outside the graph's valid
L_b x L_b x L_b block (L_b in [48, 90]), so only ~41% of the tensor
carries data.  This kernel crops to the valid blocks, cutting HBM
traffic per core from 9.4 MB (padded k-major fp8) to ~3.0 MB.

SPMD layout (one program, 8 cores, per-core data differs):
  - Graphs grouped by length L.  Each core gets one "own" graph per
    length (full block) plus a fixed 1/8 chunk-slice of each leftover
    "shared" graph, zero-padded to a fixed per-slot shape, so every
    core runs an identical instruction stream.
  - Per slot: paths cropped block, k-major [L, nch*128] fp8 (0/1 exact).
  - Scores are computed on the tensor engine: per slot, two k=128
    matmuls xT(bf16) @ Wcol(bf16) accumulate into PSUM [96, NSLOT],
    plus one k=1 ones @ b matmul for the bias.  The small xT DMA is
    issued before the big paths DMAs so scores are ready early.
  - Per 128-column chunk of a slot: one matmul with the paths chunk as
    the fp8 stationary operand [L, 128] and moving operand
    w_all[0:L, 4s:4s+4] = [sp_hi, sp_lo, ones, 0] (fp8, 2-term hi/lo
    split of the scores -> ~4e-4 rel err) -> PSUM [128, 4] =
    (num_hi, num_lo, den, -).
  - 128 chunks per PSUM bank; epilogue per bank: hi copy (scalar),
    num = hi + lo/16 (DVE), den + eps (scalar), reciprocal (DVE),
    multiply (DVE).  Output stored partition-major [128, NCH]; host
    scatters chunks back into [64, 96, 96].
"""

import sys

if "/opt/trn_rl_repo" not in sys.path:
    sys.path.insert(0, "/opt/trn_rl_repo")

import math

import ml_dtypes
import numpy as np

import concourse.bass as bass  # noqa: F401
import concourse.mybir as mybir
from concourse import bacc, bass_utils
from concourse.tile import TileContext

F32 = mybir.dt.float32
BF16 = mybir.dt.bfloat16
FP8 = mybir.dt.float8e4
AF = mybir.ActivationFunctionType

B = 64
MAX_A = 96
D = 256
N_CORES = 8
CHUNK = 128                 # stationary columns per matmul
CPT = 128                   # chunks per PSUM tile (128*4 = 512 cols = 1 bank)
EPS = 1e-8

NP_FP8 = ml_dtypes.float8_e4m3
NP_BF16 = ml_dtypes.bfloat16

_CACHE = {}


def _make_template(lengths):
    """Build the per-core slot template from the 64 graph lengths.

    Returns (slots, groups) where every core sees identical structure;
    per-core variation lives in which graph / chunk-range fills a slot.
    """
    by_len = {}
    for g, L in enumerate(lengths):
        by_len.setdefault(int(L), []).append(g)

    slots = []  # dicts: l, nch, graphs[8], chunk0[8]
    for L in sorted(by_len, reverse=True):
        gs = by_len[L]
        n_full = math.ceil(L * L / CHUNK)
        n_own = len(gs) // N_CORES
        for j in range(n_own):
            slots.append(dict(
                l=L, nch=n_full, kind="own",
                graphs=[gs[j * N_CORES + c] for c in range(N_CORES)],
                chunk0=[0] * N_CORES))
        for gsh in gs[n_own * N_CORES:]:
            m = math.ceil(n_full / N_CORES)
            slots.append(dict(
                l=L, nch=m, kind="shared",
                graphs=[gsh] * N_CORES,
                chunk0=[c * m for c in range(N_CORES)]))

    # Merge slots of equal L into one DMA group; groups ordered by
    # descending byte size so the big transfers stream first.  The DMA
    # partition count is padded to a multiple of 16: the HW DGE splits a
    # 2D transfer across (largest divisor of row count <= 16) queues, so
    # e.g. an 83-row DMA lands on a single queue at ~19 GB/s.
    groups = []  # dicts: l, lpad, slot_ids, cols
    for L in sorted(by_len, reverse=True):
        sids = [i for i, s in enumerate(slots) if s["l"] == L]
        cols = sum(slots[i]["nch"] * CHUNK for i in sids)
        groups.append(dict(l=L, lpad=16 * math.ceil(L / 16),
                           slot_ids=sids, cols=cols))
    groups.sort(key=lambda gr: -gr["l"] * gr["cols"])

    # Re-order slots into group order so matmul emission order (= PSUM
    # chunk order) matches DMA arrival order.
    order = [i for gr in groups for i in gr["slot_ids"]]
    slots = [slots[i] for i in order]
    for gr in groups:
        gr["slot_ids"] = None  # re-derive below
    pos = 0
    for gr in groups:
        n = sum(1 for s in slots if s["l"] == gr["l"])
        gr["slot_ids"] = list(range(pos, pos + n))
        pos += n

    # Column base of each slot inside its group tile, chunk base in the
    # global PSUM/output ordering, and xT column offsets.
    chunk_base = []
    acc = 0
    for s in slots:
        chunk_base.append(acc)
        acc += s["nch"]
    nch_tot = acc

    colbase = {}
    for gr in groups:
        cacc = 0
        for i in gr["slot_ids"]:
            colbase[i] = cacc
            cacc += slots[i]["nch"] * CHUNK

    xt_off = []
    xacc = 0
    for s in slots:
        xt_off.append(xacc)
        xacc += 2 * s["l"]
    # W halves and a bias column (b replicated down the partitions) ride
    # in the same tensor (their own DMAs would be 4-byte-per-row).
    w2_off = xacc
    b_off = xacc + 2
    xt_cols = 16 * math.ceil((b_off + 1) / 16)

    return dict(slots=slots, groups=groups, chunk_base=chunk_base,
                nch_tot=nch_tot, colbase=colbase, xt_off=xt_off,
                xt_cols=xt_cols, w2_off=w2_off, b_off=b_off)


def _build(lengths):
    key = tuple(int(v) for v in lengths)
    if key in _CACHE:
        return _CACHE[key]

    tmpl = _make_template(lengths)
    slots = tmpl["slots"]
    groups = tmpl["groups"]
    nslot = len(slots)
    nch_tot = tmpl["nch_tot"]

    nc = bacc.Bacc("TRN2", target_bir_lowering=False, debug=False,
                   num_devices=N_CORES)
    # The SWDGE ring (gpsimd DMAs) is unused: drop its declaration so the
    # runtime's per-queue setup/teardown sync ladder is 16 queues shorter.
    nc.m.queues = [q for q in nc.m.queues if q.name != "qPoolDynamic"]

    pg_d = [nc.dram_tensor(f"pg{i}", [gr["lpad"], gr["cols"]], FP8,
                           kind="ExternalInput")
            for i, gr in enumerate(groups)]
    xt_d = nc.dram_tensor("xT", [D // 2, tmpl["xt_cols"]], BF16,
                          kind="ExternalInput")
    out_d = nc.dram_tensor("out", [CHUNK, nch_tot], F32,
                           kind="ExternalOutput")

    with TileContext(nc) as tc:
        with (
            tc.tile_pool(name="data", bufs=1) as dpool,
            tc.tile_pool(name="psum", bufs=3, space="PSUM") as pspool,
            tc.tile_pool(name="psc", bufs=3, space="PSUM") as pscp,
            tc.tile_pool(name="epi", bufs=3) as epool,
        ):
            # ---- issue all DMAs up front.  xT (scores inputs) goes
            # first on the scalar ring (it drains ahead of sync); paths
            # groups are balanced across both rings by byte count.
            xt = dpool.tile([D // 2, tmpl["xt_cols"]], BF16, name="xt")
            nc.scalar.dma_start(out=xt[:], in_=xt_d[:])

            scalar_groups = {1, 3, 5, 6}
            gtiles = []
            for i, gr in enumerate(groups):
                gt = dpool.tile([gr["lpad"], gr["cols"]], FP8,
                                name=f"pg{i}")
                eng = nc.scalar if i in scalar_groups else nc.sync
                eng.dma_start(out=gt[:], in_=pg_d[i][:])
                gtiles.append(gt)

            w2o, bo = tmpl["w2_off"], tmpl["b_off"]
            w_all = dpool.tile([MAX_A, 4 * nslot], FP8, name="w_all")
            nc.vector.memset(w_all[:], 0.0)
            nc.vector.memset(w_all[:, 2:4 * nslot:4], 1.0)
            out_sb = dpool.tile([CHUNK, nch_tot], F32, name="out_sb")

            def emit_scores(gr):
                # per-slot scores -> PSUM, then fp8 hi/lo into w_all
                sids = gr["slot_ids"]
                ng = len(sids)
                ps_sc = pscp.tile([MAX_A, ng], F32, tag="sc")
                for j, s in enumerate(sids):
                    q, l = tmpl["xt_off"][s], slots[s]["l"]
                    nc.tensor.matmul(ps_sc[0:l, j:j + 1],
                                     lhsT=xt[:, q:q + l],
                                     rhs=xt[:, w2o:w2o + 1],
                                     start=True, stop=False)
                    nc.tensor.matmul(ps_sc[0:l, j:j + 1],
                                     lhsT=xt[:, q + l:q + 2 * l],
                                     rhs=xt[:, w2o + 1:w2o + 2],
                                     start=False, stop=True)
                wsp = epool.tile([MAX_A, 8], F32, tag="wsp")
                nc.scalar.activation(out=wsp[:, :ng], in_=ps_sc[:],
                                     func=AF.Identity,
                                     bias=xt[0:MAX_A, bo:bo + 1])
                hi = epool.tile([MAX_A, 8], FP8, tag="whi")
                nc.vector.tensor_copy(hi[:, :ng], wsp[:, :ng])
                r1 = epool.tile([MAX_A, 8], F32, tag="wr1")
                nc.vector.tensor_tensor(out=r1[:, :ng], in0=wsp[:, :ng],
                                        in1=hi[:, :ng],
                                        op=mybir.AluOpType.subtract)
                lo = epool.tile([MAX_A, 8], FP8, tag="wlo")
                nc.vector.tensor_scalar_mul(out=lo[:, :ng],
                                            in0=r1[:, :ng], scalar1=16.0)
                s0 = sids[0]
                nc.vector.tensor_copy(
                    w_all[:, 4 * s0:4 * (s0 + ng):4], hi[:, :ng])
                nc.vector.tensor_copy(
                    w_all[:, 4 * s0 + 1:4 * (s0 + ng):4], lo[:, :ng])

            def emit_epilogue(ps, t0, w):
                hi_sb = epool.tile([CHUNK, CPT], F32, tag="hi")
                nc.scalar.activation(out=hi_sb[:, :w],
                                     in_=ps[:, 0:4 * w:4], func=AF.Copy)
                numt = epool.tile([CHUNK, CPT], F32, tag="numt")
                nc.vector.scalar_tensor_tensor(
                    out=numt[:, :w], in0=ps[:, 1:4 * w:4],
                    scalar=0.0625, in1=hi_sb[:, :w],
                    op0=mybir.AluOpType.mult, op1=mybir.AluOpType.add)
                den_sb = epool.tile([CHUNK, CPT], F32, tag="den")
                nc.scalar.activation(out=den_sb[:, :w],
                                     in_=ps[:, 2:4 * w:4],
                                     func=AF.Copy, bias=EPS)
                rec = epool.tile([CHUNK, CPT], F32, tag="rec")
                nc.vector.reciprocal(out=rec[:, :w], in_=den_sb[:, :w])
                nc.vector.tensor_tensor(
                    out=out_sb[:, CPT * t0:CPT * t0 + w],
                    in0=numt[:, :w], in1=rec[:, :w],
                    op=mybir.AluOpType.mult)
                nc.sync.dma_start(
                    out=out_d[:, CPT * t0:CPT * t0 + w],
                    in_=out_sb[:, CPT * t0:CPT * t0 + w])

            # ---- main loop: scores run one group ahead of the paths
            # matmuls so the PE never stalls on the DVE hi/lo split.
            emit_scores(groups[0])
            ps = None
            n_in_tile = 0
            gidx = 0
            for gi, gr in enumerate(groups):
                if gi + 1 < len(groups):
                    emit_scores(groups[gi + 1])
                gt = gtiles[gi]
                for s in gr["slot_ids"]:
                    sl = slots[s]
                    cb = tmpl["colbase"][s]
                    l = sl["l"]
                    for c in range(sl["nch"]):
                        r = gidx % CPT
                        if r == 0:
                            n_in_tile = min(CPT, nch_tot - gidx)
                            ps = pspool.tile([CHUNK, 4 * n_in_tile], F32,
                                             tag="ps")
                        col = cb + CHUNK * c
                        nc.tensor.matmul(
                            ps[:, 4 * r:4 * r + 4],
                            lhsT=gt[0:l, col:col + CHUNK],
                            rhs=w_all[0:l, 4 * s:4 * s + 4],
                            start=True, stop=True)
                        if r == n_in_tile - 1:
                            emit_epilogue(ps, gidx // CPT, n_in_tile)
                        gidx += 1

    nc.compile()
    _CACHE[key] = (nc, tmpl)
    return nc, tmpl


def _host_prep(x, W, b, paths, lengths, offsets, tmpl):
    slots = tmpl["slots"]
    groups = tmpl["groups"]
    nslot = len(slots)

    # k-major cropped fp8 block per graph, computed once
    kmajor = {}
    for g, L in enumerate(lengths):
        if any(g in s["graphs"] for s in slots):
            blk = np.asarray(paths[g, :L, :L, :L], dtype=np.float32)
            kmajor[g] = np.ascontiguousarray(
                blk.transpose(2, 0, 1).reshape(L, L * L)).astype(NP_FP8)

    xb = np.asarray(x, dtype=np.float32).astype(NP_BF16)
    wf = np.asarray(W, np.float32)

    in_maps = []
    for core in range(N_CORES):
        im = {}
        # xT: per slot [128, 2L] = [x[:,0:128].T | x[:,128:256].T],
        # then the two W halves as columns and the bias as a row vector.
        xt = np.zeros((D // 2, tmpl["xt_cols"]), dtype=NP_BF16)
        for s, sl in enumerate(slots):
            g, L, q = sl["graphs"][core], sl["l"], tmpl["xt_off"][s]
            xg = xb[offsets[g]:offsets[g] + L]  # [L, 256]
            xt[:, q:q + L] = xg[:, :D // 2].T
            xt[:, q + L:q + 2 * L] = xg[:, D // 2:].T
        xt[:, tmpl["w2_off"]] = wf[0, :D // 2]
        xt[:, tmpl["w2_off"] + 1] = wf[0, D // 2:]
        xt[:, tmpl["b_off"]] = float(np.asarray(b)[0])
        im["xT"] = xt

        for i, gr in enumerate(groups):
            arr = np.zeros((gr["lpad"], gr["cols"]), dtype=NP_FP8)
            for s in gr["slot_ids"]:
                sl = slots[s]
                g, L = sl["graphs"][core], sl["l"]
                cb = tmpl["colbase"][s]
                c0 = sl["chunk0"][core] * CHUNK
                c1 = min(L * L, c0 + sl["nch"] * CHUNK)
                if c1 > c0:
                    arr[:L, cb:cb + (c1 - c0)] = kmajor[g][:, c0:c1]
            im[f"pg{i}"] = arr
        in_maps.append(im)
    return in_maps


LAST_RESULTS = None


def kernel(x, W, b, paths, pad_idx, _trace=False):
    global LAST_RESULTS
    pad_idx = np.asarray(pad_idx)
    lengths = np.bincount(pad_idx // MAX_A, minlength=B).astype(np.int64)
    offsets = np.zeros(B + 1, dtype=np.int64)
    np.cumsum(lengths, out=offsets[1:])

    nc, tmpl = _build(lengths)
    in_maps = _host_prep(x, W, b, paths, lengths, offsets, tmpl)
    res = bass_utils.run_bass_kernel_spmd(
        nc, in_maps, core_ids=list(range(N_CORES)), trace=_trace)
    LAST_RESULTS = res

    slots = tmpl["slots"]
    out = np.zeros((B, MAX_A, MAX_A), dtype=np.float32)
    flat = {g: np.zeros(int(L) * int(L), dtype=np.float32)
            for g, L in enumerate(lengths)}
    for core in range(N_CORES):
        oc = res.results[core]["out"]  # [128, nch_tot] partition-major
        for s, sl in enumerate(slots):
            g, L = sl["graphs"][core], sl["l"]
            cb = tmpl["chunk_base"][s]
            c0 = sl["chunk0"][core] * CHUNK
            c1 = min(L * L, c0 + sl["nch"] * CHUNK)
            if c1 > c0:
                vals = oc[:, cb:cb + sl["nch"]].T.reshape(-1)[:c1 - c0]
                flat[g][c0:c1] = vals
    for g, L in enumerate(lengths):
        L = int(L)
        out[g, :L, :L] = flat[g].reshape(L, L)
    return out
